# revision 1
# baseline (speedup 1.0000x reference)
"""Trainium2 Bass kernel for AlignShouldersToXAxis.

Math: the reference's Rodrigues construction for aligning the frame-0
shoulder vector to +X collapses to a 2D rotation in the XY plane:

    dx, dy = (p_right - p_left).xy   (frame 0, joints 6/5)
    n  = sqrt(dx^2 + dy^2);  m = max(n, 1e-12)
    cx = dx/m, cy = dy/m
    valid = (n >= 1e-6) & (|cy| >= 1e-6)
    if not valid: R = I
    out_x = cx*x + cy*y ; out_y = -cy*x + cx*y ; out_z = z

Sharding: pure data parallel, batch dim 128 -> 8 cores x 16 batches.
Per-core layout: [16, 307200] floats viewed as [(16 b x 8 k), 38400]
so partition p = b*8+k holds a contiguous 38400-float chunk of batch
b's data, and the per-batch rotation scalars are per-partition values.
"""

import numpy as np

import concourse.bacc as bacc
import concourse.mybir as mybir
from concourse.tile import TileContext
from concourse.bass_utils import run_bass_kernel_spmd

N_CORES = 8
B, T, J, C = 128, 4096, 25, 3
B_LOC = B // N_CORES            # 16 batches per core
FLAT = T * J * C                # 307200 floats per batch
K = 8                           # chunks per batch -> 16*8 = 128 partitions
F = 4800                        # floats per partition per tile (divisible by 3)

EPS = 1e-6
_f32 = mybir.dt.float32


def build(b_loc=B_LOC, flat=FLAT, k=K, f=F, io_bufs=3):
    """Build the per-core Bass program. Parameterized so tests can build a
    small variant for CoreSim."""
    assert flat % k == 0
    chunk = flat // k           # floats per partition
    assert chunk % f == 0
    n_tiles = chunk // f
    assert f % 3 == 0
    npts = f // 3
    P = b_loc * k               # partitions used (128 in prod)
    assert P <= 128

    nc = bacc.Bacc("TRN2", target_bir_lowering=False, debug=False,
                   num_devices=N_CORES)
    x = nc.dram_tensor("x", [b_loc, flat], _f32, kind="ExternalInput")
    y = nc.dram_tensor("y", [b_loc, flat], _f32, kind="ExternalOutput")
    xv = x.rearrange("b (k f) -> (b k) f", k=k)
    yv = y.rearrange("b (k f) -> (b k) f", k=k)

    mult = mybir.AluOpType.mult
    add = mybir.AluOpType.add
    is_ge = mybir.AluOpType.is_ge

    with TileContext(nc) as tc:
        with tc.tile_pool(name="scal", bufs=1) as scal, \
             tc.tile_pool(name="data", bufs=io_bufs) as data:
            # --- per-batch rotation scalars, computed redundantly on all
            # partitions of each batch (DMA-broadcast of the first 24 floats:
            # joints 5 and 6 of frame 0 live at float offsets 15..20) ---
            s24 = scal.tile([P, 24], _f32)
            for b in range(b_loc):
                nc.sync.dma_start(out=s24[b * k:(b + 1) * k, :],
                                  in_=x[b:b + 1, 0:24].to_broadcast((k, 24)))
            d2 = scal.tile([P, 2], _f32)      # (dx, dy)
            nc.vector.tensor_sub(d2, s24[:, 18:20], s24[:, 15:17])
            sq = scal.tile([P, 2], _f32)
            nc.vector.tensor_mul(sq, d2, d2)
            nsq = scal.tile([P, 1], _f32)
            nc.vector.tensor_add(nsq, sq[:, 0:1], sq[:, 1:2])
            n = scal.tile([P, 1], _f32)
            nc.scalar.sqrt(n, nsq)
            m = scal.tile([P, 1], _f32)
            nc.vector.tensor_scalar_max(m, n, 1e-12)
            r = scal.tile([P, 1], _f32)
            nc.vector.reciprocal(r, m)
            cxy = scal.tile([P, 2], _f32)     # (cx, cy)
            nc.vector.tensor_scalar(cxy, d2, r, None, mult)
            # valid = (n >= EPS) & (|cy| >= EPS)
            v1 = scal.tile([P, 1], _f32)
            nc.vector.tensor_scalar(v1, n, EPS, None, is_ge)
            acy = scal.tile([P, 1], _f32)
            nc.scalar.activation(acy, cxy[:, 1:2],
                                 mybir.ActivationFunctionType.Abs)
            v2 = scal.tile([P, 1], _f32)
            nc.vector.tensor_scalar(v2, acy, EPS, None, is_ge)
            valid = scal.tile([P, 1], _f32)
            nc.vector.tensor_mul(valid, v1, v2)
            # ccos = valid ? cx : 1 == valid*(cx-1) + 1
            # csin = valid ? cy : 0 == valid*cy
            cxm1 = scal.tile([P, 1], _f32)
            nc.vector.tensor_scalar_add(cxm1, cxy[:, 0:1], -1.0)
            ones = scal.tile([P, 1], _f32)
            nc.vector.memset(ones, 1.0)
            ccos = scal.tile([P, 1], _f32)
            nc.vector.scalar_tensor_tensor(ccos, valid, cxm1, ones, mult, add)
            csin = scal.tile([P, 1], _f32)
            nc.vector.tensor_mul(csin, valid, cxy[:, 1:2])
            ncsin = scal.tile([P, 1], _f32)
            nc.vector.tensor_scalar_mul(ncsin, csin, -1.0)

            # --- streaming rotate: in-place on the IO tile, z untouched ---
            for ti in range(n_tiles):
                tile_ = data.tile([P, f], _f32, tag="io")
                nc.sync.dma_start(out=tile_, in_=xv[:, ti * f:(ti + 1) * f])
                t3 = tile_.rearrange("p (n c) -> p n c", c=3)
                xw = t3[:, :, 0]          # [P, npts] stride-3 views
                yw = t3[:, :, 1]
                t_cy = data.tile([P, npts], _f32, tag="t_cy")
                t_cx = data.tile([P, npts], _f32, tag="t_cx")
                nc.scalar.mul(t_cy, yw, csin)     # ACT:  cy*y
                nc.scalar.mul(t_cx, xw, ncsin)    # ACT: -cy*x
                # DVE: x' = cx*x + cy*y ; y' = cx*y - cy*x  (in place)
                nc.vector.scalar_tensor_tensor(xw, xw, ccos, t_cy, mult, add)
                nc.vector.scalar_tensor_tensor(yw, yw, ccos, t_cx, mult, add)
                nc.sync.dma_start(out=yv[:, ti * f:(ti + 1) * f], in_=tile_)
    nc.compile()
    return nc


_nc_cache = None


def kernel(skeleton_seq: np.ndarray) -> np.ndarray:
    global _nc_cache
    assert skeleton_seq.shape == (B, T, J, C), skeleton_seq.shape
    if _nc_cache is None:
        _nc_cache = build()
    nc = _nc_cache
    flat = np.ascontiguousarray(skeleton_seq, dtype=np.float32).reshape(B, FLAT)
    in_maps = [{"x": flat[i * B_LOC:(i + 1) * B_LOC]} for i in range(N_CORES)]
    res = run_bass_kernel_spmd(nc, in_maps, core_ids=list(range(N_CORES)))
    out = np.concatenate([res.results[i]["y"] for i in range(N_CORES)], axis=0)
    return out.reshape(B, T, J, C)


# revision 5
# speedup vs baseline: 1.1111x; 1.1111x over previous
"""Trainium2 Bass kernel for AlignShouldersToXAxis.

Math: the reference's Rodrigues construction for aligning the frame-0
shoulder vector to +X collapses to a 2D rotation in the XY plane:

    dx, dy = (p_right - p_left).xy   (frame 0, joints 6/5)
    n  = sqrt(dx^2 + dy^2);  m = max(n, 1e-12)
    cx = dx/m, cy = dy/m
    valid = (n >= 1e-6) & (|cy| >= 1e-6)
    if not valid: R = I
    out_x = cx*x + cy*y ; out_y = -cy*x + cx*y ; out_z = z

Sharding: pure data parallel, batch dim 128 -> 8 cores x 16 batches.
Per-core layout: [16, 307200] floats viewed as [(16 b x 8 k), 38400]
so partition p = b*8+k holds a contiguous 38400-float chunk of batch
b's data, and the per-batch rotation scalars are per-partition values.
"""

import numpy as np

import concourse.bacc as bacc
import concourse.mybir as mybir
from concourse.tile import TileContext
from concourse.bass_utils import run_bass_kernel_spmd

N_CORES = 8
B, T, J, C = 128, 4096, 25, 3
B_LOC = B // N_CORES            # 16 batches per core
FLAT = T * J * C                # 307200 floats per batch
K = 8                           # chunks per batch -> 16*8 = 128 partitions
F = 2400                        # floats per partition per tile (divisible by 3)

EPS = 1e-6
_f32 = mybir.dt.float32


def build(b_loc=B_LOC, flat=FLAT, k=K, f=F, io_bufs=4):
    """Build the per-core Bass program. Parameterized so tests can build a
    small variant for CoreSim."""
    assert flat % k == 0
    chunk = flat // k           # floats per partition
    assert chunk % f == 0
    n_tiles = chunk // f
    assert f % 3 == 0
    npts = f // 3
    P = b_loc * k               # partitions used (128 in prod)
    assert P <= 128

    nc = bacc.Bacc("TRN2", target_bir_lowering=False, debug=False,
                   num_devices=N_CORES)
    x = nc.dram_tensor("x", [b_loc, flat], _f32, kind="ExternalInput")
    y = nc.dram_tensor("y", [b_loc, flat], _f32, kind="ExternalOutput")
    xv = x.rearrange("b (k f) -> (b k) f", k=k)
    yv = y.rearrange("b (k f) -> (b k) f", k=k)

    mult = mybir.AluOpType.mult
    add = mybir.AluOpType.add
    is_ge = mybir.AluOpType.is_ge

    with TileContext(nc) as tc:
        with tc.tile_pool(name="scal", bufs=1) as scal, \
             tc.tile_pool(name="data", bufs=io_bufs) as data:
            # Issue the first big tile load before anything else so the DMA
            # engines start streaming immediately; the scalar prep below
            # overlaps with it.
            tile0 = data.tile([P, f], _f32, tag="io")
            nc.sync.dma_start(out=tile0, in_=xv[:, 0:f])

            # --- per-batch rotation scalars, computed redundantly on all
            # partitions of each batch (DMA-broadcast of the first 24 floats:
            # joints 5 and 6 of frame 0 live at float offsets 15..20) ---
            s24 = scal.tile([P, 24], _f32)
            nc.sync.dma_start(
                out=s24[:],
                in_=x[:, 0:24].unsqueeze(1).to_broadcast((b_loc, k, 24)))
            d2 = scal.tile([P, 2], _f32)      # (dx, dy)
            nc.vector.tensor_sub(d2, s24[:, 18:20], s24[:, 15:17])
            sq = scal.tile([P, 2], _f32)
            nc.vector.tensor_mul(sq, d2, d2)
            nsq = scal.tile([P, 1], _f32)
            nc.vector.tensor_add(nsq, sq[:, 0:1], sq[:, 1:2])
            n = scal.tile([P, 1], _f32)
            nc.scalar.sqrt(n, nsq)
            m = scal.tile([P, 1], _f32)
            nc.vector.tensor_scalar_max(m, n, 1e-12)
            r = scal.tile([P, 1], _f32)
            nc.vector.reciprocal(r, m)
            cxy = scal.tile([P, 2], _f32)     # (cx, cy)
            nc.vector.tensor_scalar(cxy, d2, r, None, mult)
            # valid = (n >= EPS) & (|cy| >= EPS)
            v1 = scal.tile([P, 1], _f32)
            nc.vector.tensor_scalar(v1, n, EPS, None, is_ge)
            acy = scal.tile([P, 1], _f32)
            nc.scalar.activation(acy, cxy[:, 1:2],
                                 mybir.ActivationFunctionType.Abs)
            v2 = scal.tile([P, 1], _f32)
            nc.vector.tensor_scalar(v2, acy, EPS, None, is_ge)
            valid = scal.tile([P, 1], _f32)
            nc.vector.tensor_mul(valid, v1, v2)
            # ccos = valid ? cx : 1 == valid*(cx-1) + 1
            # csin = valid ? cy : 0 == valid*cy
            cxm1 = scal.tile([P, 1], _f32)
            nc.vector.tensor_scalar_add(cxm1, cxy[:, 0:1], -1.0)
            ones = scal.tile([P, 1], _f32)
            nc.vector.memset(ones, 1.0)
            ccos = scal.tile([P, 1], _f32)
            nc.vector.scalar_tensor_tensor(ccos, valid, cxm1, ones, mult, add)
            csin = scal.tile([P, 1], _f32)
            nc.vector.tensor_mul(csin, valid, cxy[:, 1:2])
            ncsin = scal.tile([P, 1], _f32)
            nc.vector.tensor_scalar_mul(ncsin, csin, -1.0)

            # --- streaming rotate: in-place on the IO tile, z untouched ---
            for ti in range(n_tiles):
                if ti == 0:
                    tile_ = tile0
                else:
                    tile_ = data.tile([P, f], _f32, tag="io")
                    nc.sync.dma_start(out=tile_,
                                      in_=xv[:, ti * f:(ti + 1) * f])
                t3 = tile_.rearrange("p (n c) -> p n c", c=3)
                xw = t3[:, :, 0]          # [P, npts] stride-3 views
                yw = t3[:, :, 1]
                t_cy = data.tile([P, npts], _f32, tag="t_cy")
                t_cx = data.tile([P, npts], _f32, tag="t_cx")
                nc.scalar.mul(t_cy, yw, csin)     # ACT:  cy*y
                nc.scalar.mul(t_cx, xw, ncsin)    # ACT: -cy*x
                # DVE: x' = cx*x + cy*y ; y' = cx*y - cy*x  (in place)
                nc.vector.scalar_tensor_tensor(xw, xw, ccos, t_cy, mult, add)
                nc.vector.scalar_tensor_tensor(yw, yw, ccos, t_cx, mult, add)
                nc.sync.dma_start(out=yv[:, ti * f:(ti + 1) * f], in_=tile_)
    nc.compile()
    return nc


_nc_cache = None


def kernel(skeleton_seq: np.ndarray) -> np.ndarray:
    global _nc_cache
    assert skeleton_seq.shape == (B, T, J, C), skeleton_seq.shape
    if _nc_cache is None:
        _nc_cache = build()
    nc = _nc_cache
    flat = np.ascontiguousarray(skeleton_seq, dtype=np.float32).reshape(B, FLAT)
    in_maps = [{"x": flat[i * B_LOC:(i + 1) * B_LOC]} for i in range(N_CORES)]
    res = run_bass_kernel_spmd(nc, in_maps, core_ids=list(range(N_CORES)))
    out = np.concatenate([res.results[i]["y"] for i in range(N_CORES)], axis=0)
    return out.reshape(B, T, J, C)
